# revision 35
# baseline (speedup 1.0000x reference)
"""AdaptiveIFSNet Trainium2 kernel.

Reference semantics: a tiny gate MLP + 2x2-matrix pipeline over K=10 IFS
parameter rows produces per-cloud `counts` summing to N. Output row j in
segment k (rows [start_k, start_k + counts_k)) is

    out[j] = w[k] @ p_in[j - start_k] + ifs_b[k]

i.e. every valid cloud applies one affine transform to a prefix of p_in.

Strategy:
- The K=10 scalar pipeline (gates, SVD clamp, counts) runs on host with
  the exact same jax ops as the reference on the CPU backend (the
  reference itself cannot run on the neuron backend - eigh has no
  lowering - so the oracle's numbers are CPU numbers). `counts` must
  match bit-for-bit: one off-by-one shifts whole segments of the gather.
- The N-point affine transform streams through all 8 NeuronCores.
  Each segment is split 8 ways so all cores share one SPMD graph; the
  2x2 matrices and biases are baked into the instruction stream as
  immediates at (runtime) compile time.
- Device layout: the per-core buffer is [128, F] partition-major; each
  piece (valid cloud) occupies a column range. DMAs move big column
  windows (piece-agnostic); compute slices piece columns out of the
  window tiles. Every chunk gets unique SBUF slots and there are at
  most 8 DMAs per core, so no instruction ever needs more than the one
  embedded semaphore wait the ISA supports.
"""

import math
import os

import numpy as np

K = 10
BASE_SIGMA = 0.5
EPS = 1e-3
TWO_PI = 2.0 * np.pi
N_CORES = 8
ALIGN = 128        # piece length granularity in points (128 pts = 2 cols/part)
WIN_COLS = 4096    # input window width in columns (4096 cols = 2 MiB)
OUT_SPLIT = 1      # output windows per input window

LAST_RESULTS = None  # BassKernelResults of the most recent device run


def _scalar_pipeline(ifs_w, ifs_b, gw1, gb1, gw2, gb2, N):
    """Replicate the reference's K=10 parameter pipeline exactly (CPU jax)."""
    import jax
    import jax.numpy as jnp

    cpu = jax.devices("cpu")[0]
    with jax.default_device(cpu):
        ifs_w = jnp.asarray(np.asarray(ifs_w))
        h = jax.nn.relu(ifs_w @ jnp.asarray(np.asarray(gw1))
                        + jnp.asarray(np.asarray(gb1)))
        gates = jax.nn.sigmoid(
            (h @ jnp.asarray(np.asarray(gw2)) + jnp.asarray(np.asarray(gb2)))[:, 0])
        mask = gates > 0.1
        fallback = jnp.arange(K) == jnp.argmax(gates)
        mask = jnp.where(mask.any(), mask, fallback)

        t1, t2, s1, s2, d1, d2 = [ifs_w[:, i] for i in range(6)]

        def _rot(theta):
            c, s = jnp.cos(theta), jnp.sin(theta)
            return jnp.stack([jnp.stack([c, -s], -1), jnp.stack([s, c], -1)], -2)

        def _diag(a, b):
            z = jnp.zeros_like(a)
            return jnp.stack([jnp.stack([a, z], -1), jnp.stack([z, b], -1)], -2)

        r1 = _rot(t1 * TWO_PI)
        r2 = _rot(t2 * TWO_PI)
        sig = _diag(jax.nn.sigmoid(s1), jax.nn.sigmoid(s2))
        d1e = jnp.sign(d1) - jax.lax.stop_gradient(d1) + d1
        d2e = jnp.sign(d2) - jax.lax.stop_gradient(d2) + d2
        dm = _diag(d1e, d2e)
        w = r1 @ sig @ r2 @ dm

        U, S, Vh = jnp.linalg.svd(w, full_matrices=False)
        S = jnp.minimum(S, 1.0 - EPS)
        w = jnp.einsum('kij,kj,kjl->kil', U, S, Vh)

        dets = jnp.abs(w[:, 0, 0] * w[:, 1, 1] - w[:, 0, 1] * w[:, 1, 0]) * mask
        probs = dets / (dets.sum() + 1e-8)
        counts = jnp.where(
            mask, jnp.maximum(jnp.round(probs * N).astype(jnp.int32), 1), 0)
        last_valid = (K - 1) - jnp.argmax(mask[::-1])
        counts = counts.at[last_valid].add(N - counts.sum())

        return (
            np.asarray(w, dtype=np.float32),
            np.asarray(counts, dtype=np.int64),
        )


def _build_device(windows, out_windows, segments, f_tot):
    """Build the SPMD Bass graph.

    windows: input windows, list of (c0, c1) absolute column ranges (one
      in-DMA each).
    out_windows: output windows, list of (parent_win_idx, oc0, oc1)
      absolute column ranges, each contained in its parent input window,
      in increasing column order (one out-DMA each). Finer than the input
      windows so output transfers start earlier.
    segments: list of (out_idx, a, b, coeffs) with [a, b) columns relative
      to the OUT window start, b-a even, coeffs = (w00, w01, w10, w11,
      bx, by).
    Per-core "pts"/"out" are [128, f_tot] f32; columns hold interleaved
    x,y data. out_x = w00*x + w01*y + bx ; out_y = w10*x + w11*y + by.
    """
    from contextlib import ExitStack

    import concourse.bass as bass
    import concourse.mybir as mybir

    f32 = mybir.dt.float32
    mult = mybir.AluOpType.mult
    add = mybir.AluOpType.add

    nc = bass.Bass()
    pts = nc.declare_dram_parameter("pts", [128, f_tot], f32, isOutput=False)
    outp = nc.declare_dram_parameter("out", [128, f_tot], f32, isOutput=True)

    segs_by_out = [[] for _ in out_windows]
    for (oi, a, b, coef) in segments:
        segs_by_out[oi].append((a, b, coef))
    n_win = len(windows)
    n_out = len(out_windows)

    # Raw Bass (no Tile): this walrus only accepts one embedded sem wait
    # per instruction, which Tile's scheduler exceeds; standalone wait_ge
    # instructions have no such limit. The pipeline is simple enough to
    # hand-schedule: unique SBUF slots for every window (the full stream
    # is staged, ~131 KB of the 224 KB partition budget), so the only
    # syncs needed are in-DMA -> compute -> out-DMA per window.
    with ExitStack() as st:
        tins = [st.enter_context(
                    nc.sbuf_tensor(f"tin{wi}", [128, c1 - c0], f32))
                for wi, (c0, c1) in enumerate(windows)]
        touts = [st.enter_context(
                    nc.sbuf_tensor(f"tout{oi}", [128, oc1 - oc0], f32))
                 for oi, (_, oc0, oc1) in enumerate(out_windows)]
        # One semaphore per input window: DMA completions on a ring are
        # not FIFO across DMAs (the 16 SDMA engines interleave), so a
        # cumulative count on a shared semaphore would race.
        in_sems = [st.enter_context(nc.semaphore(f"in_sem{wi}"))
                   for wi in range(n_win)]
        dve_sem = st.enter_context(nc.semaphore("dve_sem"))
        out_sem = st.enter_context(nc.semaphore("out_sem"))
        block = st.enter_context(nc.Block())

        @block.sync
        def _(sync):
            # Input stream on the SP HWDGE ring; no waits anywhere (slots
            # are never reused), so it prefetches at full bandwidth.
            for wi, (c0, c1) in enumerate(windows):
                sync.dma_start(out=tins[wi][:], in_=pts[:, c0:c1]) \
                    .then_inc(in_sems[wi], 16)

        @block.vector
        def _(vector):
            waited = set()
            for oi, (wi, oc0, oc1) in enumerate(out_windows):
                if wi not in waited:
                    vector.wait_ge(in_sems[wi], 16)
                    waited.add(wi)
                wc0 = windows[wi][0]
                iv = tins[wi][:].rearrange("p (n two) -> p n two", two=2)
                ov = touts[oi][:].rearrange("p (n two) -> p n two", two=2)
                off = oc0 - wc0
                last = None
                for (a, b, coef) in segs_by_out[oi]:
                    w00, w01, w10, w11, bx, by = (float(v) for v in coef)
                    X = iv[:, (off + a) // 2: (off + b) // 2, 0]
                    Y = iv[:, (off + a) // 2: (off + b) // 2, 1]
                    OX = ov[:, a // 2: b // 2, 0]
                    OY = ov[:, a // 2: b // 2, 1]
                    # out_x = (w00*x + bx) + w01*y, out_y likewise; the
                    # partial product goes straight into the output tile
                    # and the scalar_tensor_tensor adds the y-term in
                    # place.
                    nc.vector.tensor_scalar(out=OX, in0=X, scalar1=w00,
                                            scalar2=bx, op0=mult, op1=add)
                    nc.vector.tensor_scalar(out=OY, in0=X, scalar1=w10,
                                            scalar2=by, op0=mult, op1=add)
                    nc.vector.scalar_tensor_tensor(out=OX, in0=Y, scalar=w01,
                                                   in1=OX, op0=mult, op1=add)
                    last = nc.vector.scalar_tensor_tensor(
                        out=OY, in0=Y, scalar=w11, in1=OY,
                        op0=mult, op1=add)
                last.then_inc(dve_sem, 1)

        @block.scalar
        def _(scalar):
            # Output stream on the ACT HWDGE ring so it runs concurrently
            # with the input ring.
            for oi, (wi, oc0, oc1) in enumerate(out_windows):
                scalar.wait_ge(dve_sem, oi + 1)
                scalar.dma_start(out=outp[:, oc0:oc1], in_=touts[oi][:]) \
                    .then_inc(out_sem, 16)
            scalar.wait_ge(out_sem, 16 * n_out)

    return nc


def _ensure_axon_hooks_stub():
    """run_bass_kernel_spmd honors the BASS_TRACE env var and then imports
    antenv.axon_hooks, which this image lacks - pre-install a no-op stub
    (unless a real one is already registered) so a stray env var can't
    crash the kernel; tracing just degrades to off."""
    import sys
    import types
    try:
        import antenv.axon_hooks  # noqa: F401
        return
    except ImportError:
        pass
    mod = types.ModuleType("antenv.axon_hooks")
    mod.get_axon_ntff_profile_hook = lambda: None
    mod.set_axon_ntff_profile_hook = lambda h: None
    sys.modules["antenv.axon_hooks"] = mod


def kernel(**inputs):
    global LAST_RESULTS
    _ensure_axon_hooks_stub()
    from concourse.bass_utils import run_bass_kernel_spmd

    p_in = np.ascontiguousarray(np.asarray(inputs["p_in"], dtype=np.float32))
    N = p_in.shape[0]

    w, counts = _scalar_pipeline(
        inputs["ifs_w"], inputs["ifs_b"], inputs["gw1"], inputs["gb1"],
        inputs["gw2"], inputs["gb2"], N,
    )
    ifs_b = np.asarray(inputs["ifs_b"], dtype=np.float32)

    # Piece table: one piece per valid cloud, identical length on every
    # core, occupying a column range of the [128, F] per-core layout.
    csum = np.cumsum(counts)
    starts = csum - counts
    pieces = []      # (k, start_k, c_k, L, ca, cols)
    ca = 0
    for k in range(K):
        c = int(counts[k])
        if c <= 0:
            continue
        L = (c + N_CORES - 1) // N_CORES
        Lp = ((L + ALIGN - 1) // ALIGN) * ALIGN
        cols = (2 * Lp) // 128
        pieces.append((k, int(starts[k]), c, L, ca, cols))
        ca += cols
    f_tot = ca

    # Input window table (one in-DMA per window).
    windows = []
    c0 = 0
    while c0 < f_tot:
        c1 = min(c0 + WIN_COLS, f_tot)
        windows.append((c0, c1))
        c0 = c1

    # Output windows: each input window split in OUT_SPLIT pieces so the
    # output stream starts draining before a whole input window's compute
    # is done.
    out_windows = []
    for wi, (wc0, wc1) in enumerate(windows):
        width = wc1 - wc0
        n_sub = OUT_SPLIT if width >= 1024 else 1
        base = width // n_sub
        base -= base % 2
        oc0 = wc0
        for s in range(n_sub):
            oc1 = wc1 if s == n_sub - 1 else oc0 + base
            if oc1 > oc0:
                out_windows.append((wi, oc0, oc1))
            oc0 = oc1

    # Segments: piece x out-window intersections.
    segments = []
    for (k, st, c, L, pca, cols) in pieces:
        coef = (w[k, 0, 0], w[k, 0, 1], w[k, 1, 0], w[k, 1, 1],
                ifs_b[k, 0], ifs_b[k, 1])
        for oi, (wi, oc0, oc1) in enumerate(out_windows):
            a = max(pca, oc0)
            b = min(pca + cols, oc1)
            if a < b:
                segments.append((oi, a - oc0, b - oc0, coef))

    nc = _build_device(windows, out_windows, segments, f_tot)

    # Gather: per-core [128, f_tot] buffers (contiguous strip copies).
    bufs = []
    for m in range(N_CORES):
        buf = np.zeros((128, f_tot), dtype=np.float32)
        for (k, st, c, L, pca, cols) in pieces:
            lo = m * L
            hi = min((m + 1) * L, c)
            n = max(0, hi - lo)
            if n:
                strip = np.zeros((64 * cols, 2), dtype=np.float32)
                strip[:n] = p_in[lo:hi]
                buf[:, pca:pca + cols] = strip.reshape(128, cols)
        bufs.append(buf)
    in_maps = [{"pts": bufs[m]} for m in range(N_CORES)]

    res = run_bass_kernel_spmd(
        nc, in_maps, core_ids=list(range(N_CORES)),
        trace=bool(os.environ.get("BASS_TRACE")),
    )
    LAST_RESULTS = res

    # Scatter: place each core's piece outputs into the full output.
    out = np.empty((N, 2), dtype=np.float32)
    for m in range(N_CORES):
        o = res.results[m]["out"].reshape(128, f_tot)
        for (k, st, c, L, pca, cols) in pieces:
            lo = m * L
            hi = min((m + 1) * L, c)
            n = max(0, hi - lo)
            if n:
                seg = o[:, pca:pca + cols].reshape(64 * cols, 2)
                out[st + lo: st + hi] = seg[:n]
    return out


# revision 36
# speedup vs baseline: 1.0446x; 1.0446x over previous
"""AdaptiveIFSNet Trainium2 kernel.

Reference semantics: a tiny gate MLP + 2x2-matrix pipeline over K=10 IFS
parameter rows produces per-cloud `counts` summing to N. Output row j in
segment k (rows [start_k, start_k + counts_k)) is

    out[j] = w[k] @ p_in[j - start_k] + ifs_b[k]

i.e. every valid cloud applies one affine transform to a prefix of p_in.

Strategy:
- The K=10 scalar pipeline (gates, SVD clamp, counts) runs on host with
  the exact same jax ops as the reference on the CPU backend (the
  reference itself cannot run on the neuron backend - eigh has no
  lowering - so the oracle's numbers are CPU numbers). `counts` must
  match bit-for-bit: one off-by-one shifts whole segments of the gather.
- The N-point affine transform streams through all 8 NeuronCores.
  Each segment is split 8 ways so all cores share one SPMD graph; the
  2x2 matrices and biases are baked into the instruction stream as
  immediates at (runtime) compile time.
- Device layout: the per-core buffer is [128, F] partition-major; each
  piece (valid cloud) occupies a column range. DMAs move big column
  windows (piece-agnostic); compute slices piece columns out of the
  window tiles. Every chunk gets unique SBUF slots and there are at
  most 8 DMAs per core, so no instruction ever needs more than the one
  embedded semaphore wait the ISA supports.
"""

import math
import os

import numpy as np

K = 10
BASE_SIGMA = 0.5
EPS = 1e-3
TWO_PI = 2.0 * np.pi
N_CORES = 8
ALIGN = 128        # piece length granularity in points (128 pts = 2 cols/part)
WIN_COLS = 4096    # input window width in columns (4096 cols = 2 MiB)
OUT_SPLIT = 1      # output windows per input window

LAST_RESULTS = None  # BassKernelResults of the most recent device run


def _scalar_pipeline(ifs_w, ifs_b, gw1, gb1, gw2, gb2, N):
    """Replicate the reference's K=10 parameter pipeline exactly (CPU jax)."""
    import jax
    import jax.numpy as jnp

    cpu = jax.devices("cpu")[0]
    with jax.default_device(cpu):
        ifs_w = jnp.asarray(np.asarray(ifs_w))
        h = jax.nn.relu(ifs_w @ jnp.asarray(np.asarray(gw1))
                        + jnp.asarray(np.asarray(gb1)))
        gates = jax.nn.sigmoid(
            (h @ jnp.asarray(np.asarray(gw2)) + jnp.asarray(np.asarray(gb2)))[:, 0])
        mask = gates > 0.1
        fallback = jnp.arange(K) == jnp.argmax(gates)
        mask = jnp.where(mask.any(), mask, fallback)

        t1, t2, s1, s2, d1, d2 = [ifs_w[:, i] for i in range(6)]

        def _rot(theta):
            c, s = jnp.cos(theta), jnp.sin(theta)
            return jnp.stack([jnp.stack([c, -s], -1), jnp.stack([s, c], -1)], -2)

        def _diag(a, b):
            z = jnp.zeros_like(a)
            return jnp.stack([jnp.stack([a, z], -1), jnp.stack([z, b], -1)], -2)

        r1 = _rot(t1 * TWO_PI)
        r2 = _rot(t2 * TWO_PI)
        sig = _diag(jax.nn.sigmoid(s1), jax.nn.sigmoid(s2))
        d1e = jnp.sign(d1) - jax.lax.stop_gradient(d1) + d1
        d2e = jnp.sign(d2) - jax.lax.stop_gradient(d2) + d2
        dm = _diag(d1e, d2e)
        w = r1 @ sig @ r2 @ dm

        U, S, Vh = jnp.linalg.svd(w, full_matrices=False)
        S = jnp.minimum(S, 1.0 - EPS)
        w = jnp.einsum('kij,kj,kjl->kil', U, S, Vh)

        dets = jnp.abs(w[:, 0, 0] * w[:, 1, 1] - w[:, 0, 1] * w[:, 1, 0]) * mask
        probs = dets / (dets.sum() + 1e-8)
        counts = jnp.where(
            mask, jnp.maximum(jnp.round(probs * N).astype(jnp.int32), 1), 0)
        last_valid = (K - 1) - jnp.argmax(mask[::-1])
        counts = counts.at[last_valid].add(N - counts.sum())

        return (
            np.asarray(w, dtype=np.float32),
            np.asarray(counts, dtype=np.int64),
        )


def _build_device(windows, out_windows, segments, f_tot):
    """Build the SPMD Bass graph.

    windows: input windows, list of (c0, c1) absolute column ranges (one
      in-DMA each).
    out_windows: output windows, list of (parent_win_idx, oc0, oc1)
      absolute column ranges, each contained in its parent input window,
      in increasing column order (one out-DMA each). Finer than the input
      windows so output transfers start earlier.
    segments: list of (out_idx, a, b, coeffs) with [a, b) columns relative
      to the OUT window start, b-a even, coeffs = (w00, w01, w10, w11,
      bx, by).
    Per-core "pts"/"out" are [128, f_tot] f32; columns hold interleaved
    x,y data. out_x = w00*x + w01*y + bx ; out_y = w10*x + w11*y + by.
    """
    from contextlib import ExitStack

    import concourse.bass as bass
    import concourse.mybir as mybir

    f32 = mybir.dt.float32
    mult = mybir.AluOpType.mult
    add = mybir.AluOpType.add

    nc = bass.Bass()
    pts = nc.declare_dram_parameter("pts", [128, f_tot], f32, isOutput=False)
    outp = nc.declare_dram_parameter("out", [128, f_tot], f32, isOutput=True)

    segs_by_out = [[] for _ in out_windows]
    for (oi, a, b, coef) in segments:
        segs_by_out[oi].append((a, b, coef))
    n_win = len(windows)
    n_out = len(out_windows)

    # Raw Bass (no Tile): this walrus only accepts one embedded sem wait
    # per instruction, which Tile's scheduler exceeds; standalone wait_ge
    # instructions have no such limit. The pipeline is simple enough to
    # hand-schedule: unique SBUF slots for every window (the full stream
    # is staged, ~131 KB of the 224 KB partition budget), so the only
    # syncs needed are in-DMA -> compute -> out-DMA per window.
    with ExitStack() as st:
        tins = [st.enter_context(
                    nc.sbuf_tensor(f"tin{wi}", [128, c1 - c0], f32))
                for wi, (c0, c1) in enumerate(windows)]
        touts = [st.enter_context(
                    nc.sbuf_tensor(f"tout{oi}", [128, oc1 - oc0], f32))
                 for oi, (_, oc0, oc1) in enumerate(out_windows)]
        # One semaphore per input window: DMA completions on a ring are
        # not FIFO across DMAs (the 16 SDMA engines interleave), so a
        # cumulative count on a shared semaphore would race.
        in_sems = [st.enter_context(nc.semaphore(f"in_sem{wi}"))
                   for wi in range(n_win)]
        dve_sem = st.enter_context(nc.semaphore("dve_sem"))
        out_sem = st.enter_context(nc.semaphore("out_sem"))
        block = st.enter_context(nc.Block())

        @block.sync
        def _(sync):
            # Input stream on the SP HWDGE ring; no waits anywhere (slots
            # are never reused), so it prefetches at full bandwidth.
            for wi, (c0, c1) in enumerate(windows):
                sync.dma_start(out=tins[wi][:], in_=pts[:, c0:c1]) \
                    .then_inc(in_sems[wi], 16)

        @block.vector
        def _(vector):
            waited = set()
            for oi, (wi, oc0, oc1) in enumerate(out_windows):
                if wi not in waited:
                    vector.wait_ge(in_sems[wi], 16)
                    waited.add(wi)
                wc0 = windows[wi][0]
                iv = tins[wi][:].rearrange("p (n two) -> p n two", two=2)
                ov = touts[oi][:].rearrange("p (n two) -> p n two", two=2)
                off = oc0 - wc0
                last = None
                for (a, b, coef) in segs_by_out[oi]:
                    w00, w01, w10, w11, bx, by = (float(v) for v in coef)
                    X = iv[:, (off + a) // 2: (off + b) // 2, 0]
                    Y = iv[:, (off + a) // 2: (off + b) // 2, 1]
                    OX = ov[:, a // 2: b // 2, 0]
                    OY = ov[:, a // 2: b // 2, 1]
                    # out_x = (w00*x + bx) + w01*y, out_y likewise; the
                    # partial product goes straight into the output tile
                    # and the scalar_tensor_tensor adds the y-term in
                    # place.
                    nc.vector.tensor_scalar(out=OX, in0=X, scalar1=w00,
                                            scalar2=bx, op0=mult, op1=add)
                    nc.vector.tensor_scalar(out=OY, in0=X, scalar1=w10,
                                            scalar2=by, op0=mult, op1=add)
                    nc.vector.scalar_tensor_tensor(out=OX, in0=Y, scalar=w01,
                                                   in1=OX, op0=mult, op1=add)
                    last = nc.vector.scalar_tensor_tensor(
                        out=OY, in0=Y, scalar=w11, in1=OY,
                        op0=mult, op1=add)
                last.then_inc(dve_sem, 1)

        @block.scalar
        def _(scalar):
            # Output stream on the ACT HWDGE ring so it runs concurrently
            # with the input ring.
            for oi, (wi, oc0, oc1) in enumerate(out_windows):
                scalar.wait_ge(dve_sem, oi + 1)
                scalar.dma_start(out=outp[:, oc0:oc1], in_=touts[oi][:]) \
                    .then_inc(out_sem, 16)
            scalar.wait_ge(out_sem, 16 * n_out)

    return nc


def _ensure_axon_hooks_stub():
    """run_bass_kernel_spmd honors the BASS_TRACE env var and then imports
    antenv.axon_hooks, which this image lacks - pre-install a no-op stub
    (unless a real one is already registered) so a stray env var can't
    crash the kernel; tracing just degrades to off."""
    import sys
    import types
    try:
        import antenv.axon_hooks  # noqa: F401
        return
    except ImportError:
        pass
    mod = types.ModuleType("antenv.axon_hooks")
    mod.get_axon_ntff_profile_hook = lambda: None
    mod.set_axon_ntff_profile_hook = lambda h: None
    sys.modules["antenv.axon_hooks"] = mod


def kernel(**inputs):
    global LAST_RESULTS
    _ensure_axon_hooks_stub()
    from concourse.bass_utils import run_bass_kernel_spmd

    p_in = np.ascontiguousarray(np.asarray(inputs["p_in"], dtype=np.float32))
    N = p_in.shape[0]

    w, counts = _scalar_pipeline(
        inputs["ifs_w"], inputs["ifs_b"], inputs["gw1"], inputs["gb1"],
        inputs["gw2"], inputs["gb2"], N,
    )
    ifs_b = np.asarray(inputs["ifs_b"], dtype=np.float32)

    # Piece table: one piece per valid cloud, identical length on every
    # core, occupying a column range of the [128, F] per-core layout.
    csum = np.cumsum(counts)
    starts = csum - counts
    pieces = []      # (k, start_k, c_k, L, ca, cols)
    ca = 0
    for k in range(K):
        c = int(counts[k])
        if c <= 0:
            continue
        L = (c + N_CORES - 1) // N_CORES
        Lp = ((L + ALIGN - 1) // ALIGN) * ALIGN
        cols = (2 * Lp) // 128
        pieces.append((k, int(starts[k]), c, L, ca, cols))
        ca += cols
    f_tot = ca

    # Input window table (one in-DMA per window). Small head windows so
    # compute and the output stream start as early as possible (the first
    # output can only drain after window 0 is loaded and transformed);
    # full-size windows after that for DMA efficiency.
    sizes = []
    remaining = f_tot
    for s in (512, 1024):
        if remaining > 2 * s:
            sizes.append(s)
            remaining -= s
    while remaining > 0:
        s = min(WIN_COLS, remaining)
        sizes.append(s)
        remaining -= s
    windows = []
    c0 = 0
    for s in sizes:
        windows.append((c0, c0 + s))
        c0 += s

    # Output windows: each input window split in OUT_SPLIT pieces so the
    # output stream starts draining before a whole input window's compute
    # is done.
    out_windows = []
    for wi, (wc0, wc1) in enumerate(windows):
        width = wc1 - wc0
        n_sub = OUT_SPLIT if width >= 1024 else 1
        base = width // n_sub
        base -= base % 2
        oc0 = wc0
        for s in range(n_sub):
            oc1 = wc1 if s == n_sub - 1 else oc0 + base
            if oc1 > oc0:
                out_windows.append((wi, oc0, oc1))
            oc0 = oc1

    # Segments: piece x out-window intersections.
    segments = []
    for (k, st, c, L, pca, cols) in pieces:
        coef = (w[k, 0, 0], w[k, 0, 1], w[k, 1, 0], w[k, 1, 1],
                ifs_b[k, 0], ifs_b[k, 1])
        for oi, (wi, oc0, oc1) in enumerate(out_windows):
            a = max(pca, oc0)
            b = min(pca + cols, oc1)
            if a < b:
                segments.append((oi, a - oc0, b - oc0, coef))

    nc = _build_device(windows, out_windows, segments, f_tot)

    # Gather: per-core [128, f_tot] buffers (contiguous strip copies).
    bufs = []
    for m in range(N_CORES):
        buf = np.zeros((128, f_tot), dtype=np.float32)
        for (k, st, c, L, pca, cols) in pieces:
            lo = m * L
            hi = min((m + 1) * L, c)
            n = max(0, hi - lo)
            if n:
                strip = np.zeros((64 * cols, 2), dtype=np.float32)
                strip[:n] = p_in[lo:hi]
                buf[:, pca:pca + cols] = strip.reshape(128, cols)
        bufs.append(buf)
    in_maps = [{"pts": bufs[m]} for m in range(N_CORES)]

    res = run_bass_kernel_spmd(
        nc, in_maps, core_ids=list(range(N_CORES)),
        trace=bool(os.environ.get("BASS_TRACE")),
    )
    LAST_RESULTS = res

    # Scatter: place each core's piece outputs into the full output.
    out = np.empty((N, 2), dtype=np.float32)
    for m in range(N_CORES):
        o = res.results[m]["out"].reshape(128, f_tot)
        for (k, st, c, L, pca, cols) in pieces:
            lo = m * L
            hi = min((m + 1) * L, c)
            n = max(0, hi - lo)
            if n:
                seg = o[:, pca:pca + cols].reshape(64 * cols, 2)
                out[st + lo: st + hi] = seg[:n]
    return out


# revision 37
# speedup vs baseline: 1.1436x; 1.0947x over previous
"""AdaptiveIFSNet Trainium2 kernel.

Reference semantics: a tiny gate MLP + 2x2-matrix pipeline over K=10 IFS
parameter rows produces per-cloud `counts` summing to N. Output row j in
segment k (rows [start_k, start_k + counts_k)) is

    out[j] = w[k] @ p_in[j - start_k] + ifs_b[k]

i.e. every valid cloud applies one affine transform to a PREFIX of p_in.

Strategy:
- The K=10 scalar pipeline (gates, SVD clamp, counts) runs on host with
  the exact same jax ops as the reference on the CPU backend (the
  reference itself cannot run on the neuron backend - eigh has no
  lowering - so the oracle's numbers are CPU numbers). `counts` must
  match bit-for-bit: one off-by-one shifts whole segments of the gather.
- Because every segment reads a prefix of p_in, the point-range
  [m*W, (m+1)*W) of p_in (W = ceil(max_count/8)) serves core m for ALL
  clouds at once: each core loads that one contiguous ~1 MB slice and
  applies every valid cloud's affine transform to it, producing one
  output block per cloud. Input traffic is ~10x smaller than a
  per-cloud split. All cores share one SPMD graph (identical block
  sizes; coefficients baked as immediates at runtime compile); rows
  past a cloud's count are junk the host drops.
- Device: raw Bass (this walrus accepts only one embedded sem wait per
  instruction, which Tile's scheduler exceeds - standalone wait_ge has
  no limit). ScalarE computes the x-terms (w00*x+bx) straight into the
  output tiles, VectorE adds the y-terms in place, the sync engine owns
  the single input DMA plus the output DMA stream. Unique SBUF slots
  everywhere (the whole stream is staged; ~76 KB of the 224 KB
  partition budget), so the only syncs are in -> ACT -> DVE -> out.
"""

import math
import os

import numpy as np

K = 10
BASE_SIGMA = 0.5
EPS = 1e-3
TWO_PI = 2.0 * np.pi
N_CORES = 8
ALIGN = 128        # W granularity in points (128 pts = 2 cols/partition)
WIN_COLS = 4096    # output window width in columns (4096 cols = 2 MiB)

LAST_RESULTS = None  # BassKernelResults of the most recent device run


def _scalar_pipeline(ifs_w, ifs_b, gw1, gb1, gw2, gb2, N):
    """Replicate the reference's K=10 parameter pipeline exactly (CPU jax)."""
    import jax
    import jax.numpy as jnp

    cpu = jax.devices("cpu")[0]
    with jax.default_device(cpu):
        ifs_w = jnp.asarray(np.asarray(ifs_w))
        h = jax.nn.relu(ifs_w @ jnp.asarray(np.asarray(gw1))
                        + jnp.asarray(np.asarray(gb1)))
        gates = jax.nn.sigmoid(
            (h @ jnp.asarray(np.asarray(gw2)) + jnp.asarray(np.asarray(gb2)))[:, 0])
        mask = gates > 0.1
        fallback = jnp.arange(K) == jnp.argmax(gates)
        mask = jnp.where(mask.any(), mask, fallback)

        t1, t2, s1, s2, d1, d2 = [ifs_w[:, i] for i in range(6)]

        def _rot(theta):
            c, s = jnp.cos(theta), jnp.sin(theta)
            return jnp.stack([jnp.stack([c, -s], -1), jnp.stack([s, c], -1)], -2)

        def _diag(a, b):
            z = jnp.zeros_like(a)
            return jnp.stack([jnp.stack([a, z], -1), jnp.stack([z, b], -1)], -2)

        r1 = _rot(t1 * TWO_PI)
        r2 = _rot(t2 * TWO_PI)
        sig = _diag(jax.nn.sigmoid(s1), jax.nn.sigmoid(s2))
        d1e = jnp.sign(d1) - jax.lax.stop_gradient(d1) + d1
        d2e = jnp.sign(d2) - jax.lax.stop_gradient(d2) + d2
        dm = _diag(d1e, d2e)
        w = r1 @ sig @ r2 @ dm

        U, S, Vh = jnp.linalg.svd(w, full_matrices=False)
        S = jnp.minimum(S, 1.0 - EPS)
        w = jnp.einsum('kij,kj,kjl->kil', U, S, Vh)

        dets = jnp.abs(w[:, 0, 0] * w[:, 1, 1] - w[:, 0, 1] * w[:, 1, 0]) * mask
        probs = dets / (dets.sum() + 1e-8)
        counts = jnp.where(
            mask, jnp.maximum(jnp.round(probs * N).astype(jnp.int32), 1), 0)
        last_valid = (K - 1) - jnp.argmax(mask[::-1])
        counts = counts.at[last_valid].add(N - counts.sum())

        return (
            np.asarray(w, dtype=np.float32),
            np.asarray(counts, dtype=np.int64),
        )


def _build_device(w_cols, out_windows, segments, f_out):
    """Build the SPMD Bass graph.

    w_cols: width (columns) of the single shared input slice.
    out_windows: list of (oc0, oc1) absolute output column ranges (one
      out-DMA each).
    segments: list of (win_idx, a, b, ic0, coeffs): [a, b) output columns
      relative to the window, reading input columns [ic0, ic0 + (b-a));
      b-a and ic0 even; coeffs = (w00, w01, w10, w11, bx, by).
    Per-core "pts" is [128, w_cols], "out" is [128, f_out]; columns hold
    interleaved x,y. out_x = w00*x + w01*y + bx ; out_y likewise.
    """
    from contextlib import ExitStack

    import concourse.bass as bass
    import concourse.mybir as mybir

    f32 = mybir.dt.float32
    mult = mybir.AluOpType.mult
    add = mybir.AluOpType.add
    Copy = mybir.ActivationFunctionType.Copy

    nc = bass.Bass()
    pts = nc.declare_dram_parameter("pts", [128, w_cols], f32, isOutput=False)
    outp = nc.declare_dram_parameter("out", [128, f_out], f32, isOutput=True)

    segs_by_win = [[] for _ in out_windows]
    for (wi, a, b, ic0, coef) in segments:
        segs_by_win[wi].append((a, b, ic0, coef))
    n_win = len(out_windows)

    with ExitStack() as st:
        tin = st.enter_context(nc.sbuf_tensor("tin", [128, w_cols], f32))
        touts = [st.enter_context(
                    nc.sbuf_tensor(f"tout{wi}", [128, oc1 - oc0], f32))
                 for wi, (oc0, oc1) in enumerate(out_windows)]
        in_sem = st.enter_context(nc.semaphore("in_sem"))
        act_sem = st.enter_context(nc.semaphore("act_sem"))
        dve_sem = st.enter_context(nc.semaphore("dve_sem"))
        out_sem = st.enter_context(nc.semaphore("out_sem"))
        block = st.enter_context(nc.Block())

        iv_of = {}

        @block.sync
        def _(sync):
            # One input DMA, then the output stream, all on the SP HWDGE
            # ring (the input is first in the FIFO so the waiting outputs
            # never delay it).
            sync.dma_start(out=tin[:], in_=pts[:]).then_inc(in_sem, 16)
            for wi, (oc0, oc1) in enumerate(out_windows):
                sync.wait_ge(dve_sem, wi + 1)
                sync.dma_start(out=outp[:, oc0:oc1], in_=touts[wi][:]) \
                    .then_inc(out_sem, 16)
            sync.wait_ge(out_sem, 16 * n_win)

        @block.scalar
        def _(scalar):
            # ScalarE writes the x-terms (w00*x + bx, w10*x + by) straight
            # into the output tiles.
            scalar.wait_ge(in_sem, 16)
            iv = tin[:].rearrange("p (n two) -> p n two", two=2)
            for wi in range(n_win):
                ov = touts[wi][:].rearrange("p (n two) -> p n two", two=2)
                iv_of[wi] = (iv, ov)
                last = None
                for (a, b, ic0, coef) in segs_by_win[wi]:
                    w00, w01, w10, w11, bx, by = (float(v) for v in coef)
                    X = iv[:, ic0 // 2: (ic0 + b - a) // 2, 0]
                    OX = ov[:, a // 2: b // 2, 0]
                    OY = ov[:, a // 2: b // 2, 1]
                    nc.scalar.activation(out=OX, in_=X, func=Copy,
                                         bias=bx, scale=w00)
                    last = nc.scalar.activation(out=OY, in_=X, func=Copy,
                                                bias=by, scale=w10)
                last.then_inc(act_sem, 1)

        @block.vector
        def _(vector):
            # VectorE adds the y-terms in place on top of ScalarE's
            # partials: out_x += w01*y ; out_y += w11*y.
            for wi in range(n_win):
                vector.wait_ge(act_sem, wi + 1)
                iv, ov = iv_of[wi]
                last = None
                for (a, b, ic0, coef) in segs_by_win[wi]:
                    w00, w01, w10, w11, bx, by = (float(v) for v in coef)
                    Y = iv[:, ic0 // 2: (ic0 + b - a) // 2, 1]
                    OX = ov[:, a // 2: b // 2, 0]
                    OY = ov[:, a // 2: b // 2, 1]
                    nc.vector.scalar_tensor_tensor(out=OX, in0=Y, scalar=w01,
                                                   in1=OX, op0=mult, op1=add)
                    last = nc.vector.scalar_tensor_tensor(
                        out=OY, in0=Y, scalar=w11, in1=OY,
                        op0=mult, op1=add)
                last.then_inc(dve_sem, 1)

    return nc


def _ensure_axon_hooks_stub():
    """run_bass_kernel_spmd honors the BASS_TRACE env var and then imports
    antenv.axon_hooks, which this image lacks - pre-install a no-op stub
    (unless a real one is already registered) so a stray env var can't
    crash the kernel; tracing just degrades to off."""
    import sys
    import types
    try:
        import antenv.axon_hooks  # noqa: F401
        return
    except ImportError:
        pass
    mod = types.ModuleType("antenv.axon_hooks")
    mod.get_axon_ntff_profile_hook = lambda: None
    mod.set_axon_ntff_profile_hook = lambda h: None
    sys.modules["antenv.axon_hooks"] = mod


def kernel(**inputs):
    global LAST_RESULTS
    _ensure_axon_hooks_stub()
    from concourse.bass_utils import run_bass_kernel_spmd

    p_in = np.ascontiguousarray(np.asarray(inputs["p_in"], dtype=np.float32))
    N = p_in.shape[0]

    w, counts = _scalar_pipeline(
        inputs["ifs_w"], inputs["ifs_b"], inputs["gw1"], inputs["gb1"],
        inputs["gw2"], inputs["gb2"], N,
    )
    ifs_b = np.asarray(inputs["ifs_b"], dtype=np.float32)

    # Valid clouds, each producing one W-point output block per core.
    csum = np.cumsum(counts)
    starts = csum - counts
    pieces = [(k, int(starts[k]), int(counts[k]))
              for k in range(K) if counts[k] > 0]
    max_c = max(c for (_, _, c) in pieces)
    W = (max_c + N_CORES - 1) // N_CORES
    Wp = ((W + ALIGN - 1) // ALIGN) * ALIGN
    w_cols = (2 * Wp) // 128
    f_out = len(pieces) * w_cols

    # Output window table.
    out_windows = []
    c0 = 0
    while c0 < f_out:
        c1 = min(c0 + WIN_COLS, f_out)
        out_windows.append((c0, c1))
        c0 = c1

    # Segments: block x window intersections. Block vi (cloud pieces[vi])
    # occupies output columns [vi*w_cols, (vi+1)*w_cols) and reads input
    # columns [0, w_cols) at the same within-block offset.
    segments = []
    for vi, (k, st, c) in enumerate(pieces):
        coef = (w[k, 0, 0], w[k, 0, 1], w[k, 1, 0], w[k, 1, 1],
                ifs_b[k, 0], ifs_b[k, 1])
        bc0 = vi * w_cols
        for wi, (oc0, oc1) in enumerate(out_windows):
            a = max(bc0, oc0)
            b = min(bc0 + w_cols, oc1)
            if a < b:
                segments.append((wi, a - oc0, b - oc0, a - bc0, coef))

    nc = _build_device(w_cols, out_windows, segments, f_out)

    # Gather: one contiguous p_in slice per core.
    in_maps = []
    for m in range(N_CORES):
        strip = np.zeros((Wp, 2), dtype=np.float32)
        lo = m * W
        n_in = max(0, min(lo + W, N) - lo)
        if n_in:
            strip[:n_in] = p_in[lo:lo + n_in]
        in_maps.append({"pts": strip.reshape(128, w_cols)})

    res = run_bass_kernel_spmd(
        nc, in_maps, core_ids=list(range(N_CORES)),
        trace=bool(os.environ.get("BASS_TRACE")),
    )
    LAST_RESULTS = res

    # Scatter: per core, per cloud block, the valid prefix of the block.
    out = np.empty((N, 2), dtype=np.float32)
    for m in range(N_CORES):
        o = res.results[m]["out"].reshape(128, f_out)
        for vi, (k, st, c) in enumerate(pieces):
            lo = m * W
            hi = min((m + 1) * W, c)
            n = hi - lo
            if n > 0:
                blk = o[:, vi * w_cols: (vi + 1) * w_cols].reshape(Wp, 2)
                out[st + lo: st + hi] = blk[:n]
    return out


# revision 38
# speedup vs baseline: 1.3591x; 1.1885x over previous
"""AdaptiveIFSNet Trainium2 kernel.

Reference semantics: a tiny gate MLP + 2x2-matrix pipeline over K=10 IFS
parameter rows produces per-cloud `counts` summing to N. Output row j in
segment k (rows [start_k, start_k + counts_k)) is

    out[j] = w[k] @ p_in[j - start_k] + ifs_b[k]

i.e. every valid cloud applies one affine transform to a PREFIX of p_in.

Strategy:
- The K=10 scalar pipeline (gates, SVD clamp, counts) runs on host with
  the exact same jax ops as the reference on the CPU backend (the
  reference itself cannot run on the neuron backend - eigh has no
  lowering - so the oracle's numbers are CPU numbers). `counts` must
  match bit-for-bit: one off-by-one shifts whole segments of the gather.
- Because every segment reads a prefix of p_in, the point-range
  [m*W, (m+1)*W) of p_in (W = ceil(max_count/8)) serves core m for ALL
  clouds at once: each core loads that one contiguous ~1 MB slice and
  applies every valid cloud's affine transform to it, producing one
  output block per cloud. Input traffic is ~10x smaller than a
  per-cloud split. All cores share one SPMD graph (identical block
  sizes; coefficients baked as immediates at runtime compile); rows
  past a cloud's count are junk the host drops.
- Device: raw Bass (this walrus accepts only one embedded sem wait per
  instruction, which Tile's scheduler exceeds - standalone wait_ge has
  no limit). ScalarE computes the x-terms (w00*x+bx) straight into the
  output tiles, VectorE adds the y-terms in place, the sync engine owns
  the single input DMA plus the output DMA stream. Unique SBUF slots
  everywhere (the whole stream is staged; ~76 KB of the 224 KB
  partition budget), so the only syncs are in -> ACT -> DVE -> out.
"""

import math
import os

import numpy as np

K = 10
BASE_SIGMA = 0.5
EPS = 1e-3
TWO_PI = 2.0 * np.pi
N_CORES = 8
ALIGN = 128        # W granularity in points (128 pts = 2 cols/partition)
WIN_COLS = 2048    # output window width in columns (2048 cols = 1 MiB)

LAST_RESULTS = None  # BassKernelResults of the most recent device run


def _scalar_pipeline(ifs_w, ifs_b, gw1, gb1, gw2, gb2, N):
    """Replicate the reference's K=10 parameter pipeline exactly (CPU jax)."""
    import jax
    import jax.numpy as jnp

    cpu = jax.devices("cpu")[0]
    with jax.default_device(cpu):
        ifs_w = jnp.asarray(np.asarray(ifs_w))
        h = jax.nn.relu(ifs_w @ jnp.asarray(np.asarray(gw1))
                        + jnp.asarray(np.asarray(gb1)))
        gates = jax.nn.sigmoid(
            (h @ jnp.asarray(np.asarray(gw2)) + jnp.asarray(np.asarray(gb2)))[:, 0])
        mask = gates > 0.1
        fallback = jnp.arange(K) == jnp.argmax(gates)
        mask = jnp.where(mask.any(), mask, fallback)

        t1, t2, s1, s2, d1, d2 = [ifs_w[:, i] for i in range(6)]

        def _rot(theta):
            c, s = jnp.cos(theta), jnp.sin(theta)
            return jnp.stack([jnp.stack([c, -s], -1), jnp.stack([s, c], -1)], -2)

        def _diag(a, b):
            z = jnp.zeros_like(a)
            return jnp.stack([jnp.stack([a, z], -1), jnp.stack([z, b], -1)], -2)

        r1 = _rot(t1 * TWO_PI)
        r2 = _rot(t2 * TWO_PI)
        sig = _diag(jax.nn.sigmoid(s1), jax.nn.sigmoid(s2))
        d1e = jnp.sign(d1) - jax.lax.stop_gradient(d1) + d1
        d2e = jnp.sign(d2) - jax.lax.stop_gradient(d2) + d2
        dm = _diag(d1e, d2e)
        w = r1 @ sig @ r2 @ dm

        U, S, Vh = jnp.linalg.svd(w, full_matrices=False)
        S = jnp.minimum(S, 1.0 - EPS)
        w = jnp.einsum('kij,kj,kjl->kil', U, S, Vh)

        dets = jnp.abs(w[:, 0, 0] * w[:, 1, 1] - w[:, 0, 1] * w[:, 1, 0]) * mask
        probs = dets / (dets.sum() + 1e-8)
        counts = jnp.where(
            mask, jnp.maximum(jnp.round(probs * N).astype(jnp.int32), 1), 0)
        last_valid = (K - 1) - jnp.argmax(mask[::-1])
        counts = counts.at[last_valid].add(N - counts.sum())

        return (
            np.asarray(w, dtype=np.float32),
            np.asarray(counts, dtype=np.int64),
        )


def _build_device(w_cols, out_windows, segments, f_out):
    """Build the SPMD Bass graph.

    w_cols: width (columns) of the single shared input slice.
    out_windows: list of (oc0, oc1) absolute output column ranges (one
      out-DMA each).
    segments: list of (win_idx, a, b, ic0, coeffs): [a, b) output columns
      relative to the window, reading input columns [ic0, ic0 + (b-a));
      b-a and ic0 even; coeffs = (w00, w01, w10, w11, bx, by).
    Per-core "pts" is [128, w_cols], "out" is [128, f_out]; columns hold
    interleaved x,y. out_x = w00*x + w01*y + bx ; out_y likewise.
    """
    from contextlib import ExitStack

    import concourse.bass as bass
    import concourse.mybir as mybir

    f32 = mybir.dt.float32
    mult = mybir.AluOpType.mult
    add = mybir.AluOpType.add
    Copy = mybir.ActivationFunctionType.Copy

    nc = bass.Bass()
    pts = nc.declare_dram_parameter("pts", [128, w_cols], f32, isOutput=False)
    outp = nc.declare_dram_parameter("out", [128, f_out], f32, isOutput=True)

    segs_by_win = [[] for _ in out_windows]
    for (wi, a, b, ic0, coef) in segments:
        segs_by_win[wi].append((a, b, ic0, coef))
    n_win = len(out_windows)

    with ExitStack() as st:
        tin = st.enter_context(nc.sbuf_tensor("tin", [128, w_cols], f32))
        touts = [st.enter_context(
                    nc.sbuf_tensor(f"tout{wi}", [128, oc1 - oc0], f32))
                 for wi, (oc0, oc1) in enumerate(out_windows)]
        in_sem = st.enter_context(nc.semaphore("in_sem"))
        act_sem = st.enter_context(nc.semaphore("act_sem"))
        dve_sem = st.enter_context(nc.semaphore("dve_sem"))
        out_sem = st.enter_context(nc.semaphore("out_sem"))
        block = st.enter_context(nc.Block())

        iv_of = {}

        @block.sync
        def _(sync):
            # One input DMA, then the output stream, all on the SP HWDGE
            # ring (the input is first in the FIFO so the waiting outputs
            # never delay it).
            sync.dma_start(out=tin[:], in_=pts[:]).then_inc(in_sem, 16)
            for wi, (oc0, oc1) in enumerate(out_windows):
                sync.wait_ge(dve_sem, wi + 1)
                sync.dma_start(out=outp[:, oc0:oc1], in_=touts[wi][:]) \
                    .then_inc(out_sem, 16)
            sync.wait_ge(out_sem, 16 * n_win)

        @block.scalar
        def _(scalar):
            # ScalarE writes the x-terms (w00*x + bx, w10*x + by) straight
            # into the output tiles.
            scalar.wait_ge(in_sem, 16)
            iv = tin[:].rearrange("p (n two) -> p n two", two=2)
            for wi in range(n_win):
                ov = touts[wi][:].rearrange("p (n two) -> p n two", two=2)
                iv_of[wi] = (iv, ov)
                last = None
                for (a, b, ic0, coef) in segs_by_win[wi]:
                    w00, w01, w10, w11, bx, by = (float(v) for v in coef)
                    X = iv[:, ic0 // 2: (ic0 + b - a) // 2, 0]
                    OX = ov[:, a // 2: b // 2, 0]
                    OY = ov[:, a // 2: b // 2, 1]
                    nc.scalar.activation(out=OX, in_=X, func=Copy,
                                         bias=bx, scale=w00)
                    last = nc.scalar.activation(out=OY, in_=X, func=Copy,
                                                bias=by, scale=w10)
                last.then_inc(act_sem, 1)

        @block.vector
        def _(vector):
            # VectorE adds the y-terms in place on top of ScalarE's
            # partials: out_x += w01*y ; out_y += w11*y.
            for wi in range(n_win):
                vector.wait_ge(act_sem, wi + 1)
                iv, ov = iv_of[wi]
                last = None
                for (a, b, ic0, coef) in segs_by_win[wi]:
                    w00, w01, w10, w11, bx, by = (float(v) for v in coef)
                    Y = iv[:, ic0 // 2: (ic0 + b - a) // 2, 1]
                    OX = ov[:, a // 2: b // 2, 0]
                    OY = ov[:, a // 2: b // 2, 1]
                    nc.vector.scalar_tensor_tensor(out=OX, in0=Y, scalar=w01,
                                                   in1=OX, op0=mult, op1=add)
                    last = nc.vector.scalar_tensor_tensor(
                        out=OY, in0=Y, scalar=w11, in1=OY,
                        op0=mult, op1=add)
                last.then_inc(dve_sem, 1)

    return nc


def _ensure_axon_hooks_stub():
    """run_bass_kernel_spmd honors the BASS_TRACE env var and then imports
    antenv.axon_hooks, which this image lacks - pre-install a no-op stub
    (unless a real one is already registered) so a stray env var can't
    crash the kernel; tracing just degrades to off."""
    import sys
    import types
    try:
        import antenv.axon_hooks  # noqa: F401
        return
    except ImportError:
        pass
    mod = types.ModuleType("antenv.axon_hooks")
    mod.get_axon_ntff_profile_hook = lambda: None
    mod.set_axon_ntff_profile_hook = lambda h: None
    sys.modules["antenv.axon_hooks"] = mod


def kernel(**inputs):
    global LAST_RESULTS
    _ensure_axon_hooks_stub()
    from concourse.bass_utils import run_bass_kernel_spmd

    p_in = np.ascontiguousarray(np.asarray(inputs["p_in"], dtype=np.float32))
    N = p_in.shape[0]

    w, counts = _scalar_pipeline(
        inputs["ifs_w"], inputs["ifs_b"], inputs["gw1"], inputs["gb1"],
        inputs["gw2"], inputs["gb2"], N,
    )
    ifs_b = np.asarray(inputs["ifs_b"], dtype=np.float32)

    # Valid clouds, each producing one W-point output block per core.
    csum = np.cumsum(counts)
    starts = csum - counts
    pieces = [(k, int(starts[k]), int(counts[k]))
              for k in range(K) if counts[k] > 0]
    max_c = max(c for (_, _, c) in pieces)
    W = (max_c + N_CORES - 1) // N_CORES
    Wp = ((W + ALIGN - 1) // ALIGN) * ALIGN
    w_cols = (2 * Wp) // 128
    f_out = len(pieces) * w_cols

    # Output window table.
    out_windows = []
    c0 = 0
    while c0 < f_out:
        c1 = min(c0 + WIN_COLS, f_out)
        out_windows.append((c0, c1))
        c0 = c1

    # Segments: block x window intersections. Block vi (cloud pieces[vi])
    # occupies output columns [vi*w_cols, (vi+1)*w_cols) and reads input
    # columns [0, w_cols) at the same within-block offset.
    segments = []
    for vi, (k, st, c) in enumerate(pieces):
        coef = (w[k, 0, 0], w[k, 0, 1], w[k, 1, 0], w[k, 1, 1],
                ifs_b[k, 0], ifs_b[k, 1])
        bc0 = vi * w_cols
        for wi, (oc0, oc1) in enumerate(out_windows):
            a = max(bc0, oc0)
            b = min(bc0 + w_cols, oc1)
            if a < b:
                segments.append((wi, a - oc0, b - oc0, a - bc0, coef))

    nc = _build_device(w_cols, out_windows, segments, f_out)

    # Gather: one contiguous p_in slice per core.
    in_maps = []
    for m in range(N_CORES):
        strip = np.zeros((Wp, 2), dtype=np.float32)
        lo = m * W
        n_in = max(0, min(lo + W, N) - lo)
        if n_in:
            strip[:n_in] = p_in[lo:lo + n_in]
        in_maps.append({"pts": strip.reshape(128, w_cols)})

    res = run_bass_kernel_spmd(
        nc, in_maps, core_ids=list(range(N_CORES)),
        trace=bool(os.environ.get("BASS_TRACE")),
    )
    LAST_RESULTS = res

    # Scatter: per core, per cloud block, the valid prefix of the block.
    out = np.empty((N, 2), dtype=np.float32)
    for m in range(N_CORES):
        o = res.results[m]["out"].reshape(128, f_out)
        for vi, (k, st, c) in enumerate(pieces):
            lo = m * W
            hi = min((m + 1) * W, c)
            n = hi - lo
            if n > 0:
                blk = o[:, vi * w_cols: (vi + 1) * w_cols].reshape(Wp, 2)
                out[st + lo: st + hi] = blk[:n]
    return out
